# revision 4
# baseline (speedup 1.0000x reference)
"""BWGNN (Beta-Wavelet GNN) forward on 8 Trainium2 NeuronCores.

v2: column-major dense phases (bf16 matmuls, fused bias+relu on DVE, paired
TensorE transposes), chunk-major gather emission with skewed quarter-realigns
on the 4 SWDGE queues, idx tiles resident across both hops.

Nodes are partitioned across 8 cores (12500 each); dense linears are
data-parallel. Each polynomial hop: scale rows by d^-1/2, AllGather the scaled
table, bulk random gather of in-edge src rows with dma_gather (int16 indices
against 4 src-range chunks, one SWDGE queue per call round-robin), strided
vector reduce per 128-node block, realign gather, fused epilogue.

kernel(**inputs) takes FULL inputs and returns the FULL [N, 2] output.
"""
import os
import numpy as np

LAST_EXEC_NS = None

N = 100000
E = 1600000
IN = 128
H = 64
C = 2
THETAS = [[3.0, -3.0, 0.75], [0.0, 3.0, -1.5], [0.0, 0.0, 0.75]]

M = 8            # cores
NL = N // M      # 12500 nodes per core
NP = 128
NBLK = (NL + NP - 1) // NP   # 98
NPAD = NBLK * NP             # 12544
TRr = NPAD + 1               # per-rank table rows (zero row at NPAD)
NCHUNK = 4
CHROWS = 2 * TRr             # table rows per chunk (2 ranks) = 25002 < 32768
PADIDX = NPAD                # chunk-relative row of the first rank's zero row
MAX_IDX_CALL = 4096
MAX_NB = 16                  # max blocks per gather call (bounds reduce tile)
PB = [0, 24, 48, 73, 98]     # realign quarter-part block boundaries


def _wrap_idx(flat):
    """int16 flat gather list -> [128, len/16] SBUF wrap (16 partitions, x8)."""
    iw = len(flat) // 16
    w = flat.reshape(iw, 16).T
    return np.ascontiguousarray(np.tile(w, (8, 1)).astype(np.int16))


def _host_prep(in_feat, src, dst):
    deg = np.bincount(dst, minlength=N)
    dinv = (1.0 / np.sqrt(np.maximum(deg, 1))).astype(np.float32)

    core_of = dst // NL
    chunk_of = src // (2 * NL)
    idx16 = ((src // NL - 2 * chunk_of) * TRr + src % NL).astype(np.int32)

    key = core_of * NCHUNK + chunk_of
    order = np.argsort(key, kind="stable")
    bounds = np.searchsorted(key[order], np.arange(M * NCHUNK + 1))

    K = np.zeros((NCHUNK, NBLK), dtype=np.int64)
    groups = {}
    degc_all = np.zeros((M, NCHUNK, NPAD), dtype=np.int64)
    ords = np.empty((M, NCHUNK, NPAD), dtype=np.int64)
    lanes = np.empty((M, NCHUNK, NPAD), dtype=np.int32)
    for c in range(M):
        for c4 in range(NCHUNK):
            g = order[bounds[c * NCHUNK + c4] : bounds[c * NCHUNK + c4 + 1]]
            groups[(c, c4)] = g
            dl = dst[g] - c * NL
            dc = np.bincount(dl, minlength=NPAD)
            degc_all[c, c4] = dc
            o = np.argsort(-dc, kind="stable")
            ords[c, c4] = o
            inv = np.empty(NPAD, dtype=np.int32)
            inv[o] = np.arange(NPAD, dtype=np.int32)
            lanes[c, c4] = inv
            K[c4] = np.maximum(K[c4], dc[o].reshape(NBLK, NP)[:, 0])

    # call schedule: per chunk, runs of equal-K consecutive blocks, capped
    calls = []  # (c4, kb, b0, nb, nidx)
    for c4 in range(NCHUNK):
        b = 0
        while b < NBLK:
            kb = int(K[c4][b])
            if kb == 0:
                b += 1
                continue
            e_ = b
            while e_ + 1 < NBLK and int(K[c4][e_ + 1]) == kb:
                e_ += 1
            maxnb = min(MAX_NB, max(1, MAX_IDX_CALL // (NP * kb)))
            while b <= e_:
                nb = min(maxnb, e_ - b + 1)
                calls.append((c4, kb, b, nb, NP * kb * nb))
                b += nb

    chunk_cols = [0] * NCHUNK
    call_col = []
    for (c4, kb, b0, nb, nidx) in calls:
        call_col.append(chunk_cols[c4])
        chunk_cols[c4] += nidx // 16

    idx_inputs = []
    ridx_inputs = []
    for c in range(M):
        per_chunk = []
        for c4 in range(NCHUNK):
            g = groups[(c, c4)]
            dl = dst[g] - c * NL
            lane = lanes[c, c4][dl].astype(np.int64)
            eorder = np.argsort(lane, kind="stable")
            ge = g[eorder]
            lane_s = lane[eorder]
            counts = degc_all[c, c4][ords[c, c4]]
            starts = np.zeros(NPAD + 1, dtype=np.int64)
            np.cumsum(counts, out=starts[1:])
            slot = np.arange(len(ge)) - starts[lane_s]
            flat = np.full(chunk_cols[c4] * 16, PADIDX, dtype=np.int32)
            blk = lane_s // NP
            j = lane_s % NP
            for ci, (cc4, kb, b0, nb, nidx) in enumerate(calls):
                if cc4 != c4:
                    continue
                sel = (blk >= b0) & (blk < b0 + nb) & (slot < kb)
                base = call_col[ci] * 16
                pos = base + ((blk[sel] - b0) * kb + slot[sel]) * NP + j[sel]
                flat[pos] = idx16[ge[sel]]
            per_chunk.append(_wrap_idx(flat.astype(np.int16)))
        idx_inputs.append(per_chunk)
        rflat = np.concatenate(
            [lanes[c, c4][:NPAD].astype(np.int16) for c4 in range(NCHUNK)]
        )
        ridx_inputs.append(_wrap_idx(rflat))

    xt_in, dinv_in = [], []
    for c in range(M):
        xt = np.zeros((IN, NPAD), dtype=np.float32)
        xt[:, :NL] = in_feat[c * NL : (c + 1) * NL].T
        xt_in.append(np.ascontiguousarray(xt))
        dv = np.ones(NPAD, dtype=np.float32)
        dv[:NL] = dinv[c * NL : (c + 1) * NL]
        dinv_in.append(np.ascontiguousarray(dv.reshape(NBLK, NP).T))
    return calls, call_col, chunk_cols, K, idx_inputs, ridx_inputs, xt_in, dinv_in


def _weights(W1, b1, W2, b2, W3, b3, W4, b4):
    Mk = [
        sum(THETAS[t][k] * W3[:, t * H : (t + 1) * H] for t in range(len(THETAS)))
        for k in range(3)
    ]
    return {
        "W1t": np.ascontiguousarray(W1.T.astype(np.float32)),
        "W2t": np.ascontiguousarray(W2.T.astype(np.float32)),
        "M0t": np.ascontiguousarray(Mk[0].T.astype(np.float32)),
        "M1t": np.ascontiguousarray(Mk[1].T.astype(np.float32)),
        "M2t": np.ascontiguousarray(Mk[2].T.astype(np.float32)),
        "W4t": np.ascontiguousarray(W4.T.astype(np.float32)),
        "b1c": b1.reshape(H, 1).astype(np.float32),
        "b2c": b2.reshape(H, 1).astype(np.float32),
        "b3c": b3.reshape(H, 1).astype(np.float32),
        "b4c": b4.reshape(C, 1).astype(np.float32),
    }


def _build_program(calls, call_col, chunk_cols, K):
    import concourse.bacc as bacc
    import concourse.mybir as mybir
    import concourse.tile as tile
    from concourse.library_config import mlp
    from concourse.masks import make_identity

    f32 = mybir.dt.float32
    bf16 = mybir.dt.bfloat16
    i16 = mybir.dt.int16
    AX = mybir.AxisListType
    OP = mybir.AluOpType

    nc = bacc.Bacc(
        "TRN2", target_bir_lowering=False, debug=False, num_devices=M,
        num_swdge_queues=4,
    )

    xt_ext = nc.declare_dram_parameter("xt", [IN, NPAD], f32, isOutput=False)
    dinv_ext = nc.declare_dram_parameter("dinvT", [NP, NBLK], f32, isOutput=False)
    idx_ext = [
        nc.declare_dram_parameter(f"idx{c4}", [128, chunk_cols[c4]], i16, isOutput=False)
        for c4 in range(NCHUNK)
    ]
    ridx_ext = nc.declare_dram_parameter(
        "ridx", [128, NCHUNK * (NPAD // 16)], i16, isOutput=False
    )
    wshapes = [
        ("W1t", [IN, H], f32), ("W2t", [H, H], f32), ("M0t", [H, H], f32),
        ("M1t", [H, H], f32), ("M2t", [H, H], f32), ("W4t", [H, C], f32),
        ("b1c", [H, 1], f32), ("b2c", [H, 1], f32), ("b3c", [H, 1], f32),
        ("b4c", [C, 1], f32),
    ]
    wext = {nm: nc.declare_dram_parameter(nm, s, dt, isOutput=False) for nm, s, dt in wshapes}
    out_ext = nc.declare_dram_parameter("out", [C, NPAD], f32, isOutput=True)

    gloc = [nc.dram_tensor(f"g{h}loc", [TRr, H], f32) for h in range(2)]
    gfull = [
        nc.dram_tensor(f"g{h}full", [M * TRr, H], f32, addr_space="Shared")
        for h in range(2)
    ]
    aggdram = [nc.dram_tensor(f"agg{c4}", [NPAD, H], f32) for c4 in range(NCHUNK)]
    rg = [list(range(M))]

    with tile.TileContext(nc) as tc:
        with (
            tc.tile_pool(name="const", bufs=1) as cpool,
            tc.tile_pool(name="big", bufs=1) as bigpool,
            tc.tile_pool(name="xt", bufs=2) as xtpool,
            tc.tile_pool(name="work", bufs=3) as wpool,
            tc.tile_pool(name="idxp", bufs=1) as ipool,
            tc.tile_pool(name="gath", bufs=4) as gpool,
            tc.tile_pool(name="ridxp", bufs=2) as rpool,
            tc.tile_pool(name="ps", bufs=3, space="PSUM") as pspool,
        ):
            nc.gpsimd.load_library(mlp)

            W = {}
            for nm, s, dt in wshapes:
                W[nm] = cpool.tile(list(s), dt, tag=nm, name=nm)
                nc.sync.dma_start(out=W[nm][:], in_=wext[nm][:])
            dinvT = cpool.tile([NP, NBLK], f32, tag="dinvT")
            nc.sync.dma_start(out=dinvT[:], in_=dinv_ext[:])
            zrow = cpool.tile([1, H], f32, tag="zrow")
            nc.vector.memset(zrow[:], 0.0)
            zblk = cpool.tile([NP, H], f32, tag="zblk")
            nc.vector.memset(zblk[:], 0.0)
            ident = cpool.tile([NP, NP], f32, tag="ident")
            make_identity(nc, ident[:])
            ridx_t = cpool.tile([128, NCHUNK * (NPAD // 16)], i16, tag="ridx")
            nc.sync.dma_start(out=ridx_t[:], in_=ridx_ext[:])

            f0 = bigpool.tile([NP, NBLK * H], f32, tag="f0")
            f1 = bigpool.tile([NP, NBLK * H], f32, tag="f1")
            fX = bigpool.tile([NP, NBLK * H], f32, tag="fX")

            gloc_v = [g.ap()[0:NPAD, :].rearrange("(b j) d -> j b d", j=NP) for g in gloc]
            agg_v = [a.ap().rearrange("(b j) d -> j b d", j=NP) for a in aggdram]
            dbc = (
                dinvT[:]
                .rearrange("p (b o) -> p b o", o=1)
                .to_broadcast([NP, NBLK, H])
            )

            # ---------- phase A: L1 + L2 column-major, flip back per block ----
            GA = 4
            for g4 in range(0, NBLK, GA):
                nbg = min(GA, NBLK - g4)
                wg = nbg * NP
                xt = xtpool.tile([IN, GA * NP], f32, tag="xt")
                nc.sync.dma_start(
                    out=xt[:, :wg], in_=xt_ext[:, g4 * NP : g4 * NP + wg]
                )
                ps1 = pspool.tile([NP, 512], f32, tag="pS", space="PSUM")
                nc.tensor.matmul(out=ps1[:H, :wg], lhsT=W["W1t"][:], rhs=xt[:, :wg], start=True, stop=True)
                h1T = wpool.tile([H, GA * NP], f32, tag="h1T")
                nc.vector.tensor_scalar(
                    out=h1T[:, :wg], in0=ps1[:H, :wg],
                    scalar1=W["b1c"][:, 0:1], scalar2=0.0, op0=OP.add, op1=OP.max,
                )
                ps2 = pspool.tile([NP, 512], f32, tag="pS", space="PSUM")
                nc.tensor.matmul(out=ps2[:H, :wg], lhsT=W["W2t"][:], rhs=h1T[:, :wg], start=True, stop=True)
                f0T = wpool.tile([H, GA * NP], f32, tag="f0T")
                nc.vector.tensor_scalar(
                    out=f0T[:, :wg], in0=ps2[:H, :wg],
                    scalar1=W["b2c"][:, 0:1], scalar2=0.0, op0=OP.add, op1=OP.max,
                )
                gb = wpool.tile([NP, GA * H], f32, tag="gb")
                for i in range(nbg):
                    b = g4 + i
                    psT = pspool.tile([NP, NP], f32, tag="pA", space="PSUM")
                    nc.tensor.transpose(
                        out=psT[:, :H], in_=f0T[:, i * NP : (i + 1) * NP],
                        identity=ident[:H, :H],
                    )
                    f0b = f0[:, b * H : (b + 1) * H]
                    nc.vector.tensor_copy(f0b, psT[:, :H])
                    nc.vector.tensor_scalar(
                        out=gb[:, i * H : (i + 1) * H], in0=f0b,
                        scalar1=dinvT[:, b : b + 1], scalar2=None, op0=OP.mult,
                    )
                nc.scalar.dma_start(
                    out=gloc_v[0][:, g4 : g4 + nbg, :],
                    in_=gb[:, : nbg * H].rearrange("p (b d) -> p b d", b=nbg),
                )
            nc.sync.dma_start(out=gloc[0][NPAD : NPAD + 1, :], in_=zrow[:])

            nc.gpsimd.collective_compute(
                "AllGather", OP.bypass, replica_groups=rg,
                ins=[gloc[0].ap().opt()], outs=[gfull[0].ap().opt()],
            )

            # load all idx tiles once (both hops reuse them)
            cur_idx = {}
            for c4 in range(NCHUNK):
                it = ipool.tile([128, chunk_cols[c4]], i16, tag=f"idxc{c4}")
                nc.sync.dma_start(out=it[:], in_=idx_ext[c4][:])
                cur_idx[c4] = it

            by_chunk = [[] for _ in range(NCHUNK)]
            for ci, cl in enumerate(calls):
                by_chunk[cl[0]].append((ci, cl))

            # ---------- two hops ----------
            for h in range(2):
                fprev = f0 if h == 0 else f1
                fout = f1 if h == 0 else fX
                table = gfull[h]

                def emit_realign(c4):
                    for p in range(4):
                        pb0, pb1 = PB[p], PB[p + 1]
                        npb = pb1 - pb0
                        rt = rpool.tile([NP, 25, H], f32, tag="rt")
                        coff = c4 * (NPAD // 16) + pb0 * (NP // 16)
                        nc.gpsimd.dma_gather(
                            rt[:, :npb, :],
                            aggdram[c4][:, :],
                            ridx_t[:, coff : coff + npb * (NP // 16)],
                            npb * NP,
                            npb * NP,
                            H,
                            single_packet=False,
                            queue_num=p,
                        )
                        rtf = rt[:, :npb, :].rearrange("p b d -> p (b d)")
                        fxs = fX[:, pb0 * H : pb1 * H]
                        if c4 == 0:
                            nc.vector.tensor_copy(fxs, rtf)
                        else:
                            nc.vector.tensor_tensor(out=fxs, in0=fxs, in1=rtf, op=OP.add)

                qrr = 0
                for c4 in range(NCHUNK):
                    for ci, (cc4, kb, b0, nb, nidx) in by_chunk[c4]:
                        S = nidx // NP
                        dst_t = gpool.tile([NP, MAX_IDX_CALL // NP, H], f32, tag="gdst")
                        nc.gpsimd.dma_gather(
                            dst_t[:, :S, :],
                            table[c4 * CHROWS : (c4 + 1) * CHROWS, :],
                            cur_idx[c4][:, call_col[ci] : call_col[ci] + nidx // 16],
                            nidx,
                            nidx,
                            H,
                            single_packet=False,
                            queue_num=qrr % 4,
                        )
                        qrr += 1
                        red = wpool.tile([NP, MAX_NB * H], f32, tag="red")
                        nc.vector.tensor_reduce(
                            out=red[:, : nb * H].rearrange("p (b d) -> p b d", b=nb),
                            in_=dst_t[:, :S, :].rearrange("p (b k) d -> p b d k", b=nb, k=kb),
                            axis=AX.X,
                            op=OP.add,
                        )
                        nc.scalar.dma_start(
                            out=agg_v[c4][:, b0 : b0 + nb, :],
                            in_=red[:, : nb * H].rearrange("p (b d) -> p b d", b=nb),
                        )
                    for b in range(NBLK):
                        if K[c4][b] == 0:
                            nc.sync.dma_start(
                                out=aggdram[c4][b * NP : (b + 1) * NP, :], in_=zblk[:]
                            )
                    if c4 >= 1:
                        emit_realign(c4 - 1)
                emit_realign(NCHUNK - 1)

                fX3 = fX[:].rearrange("p (b d) -> p b d", b=NBLK)
                nc.vector.tensor_tensor(out=fX3, in0=fX3, in1=dbc, op=OP.mult)
                nc.vector.tensor_tensor(out=fout[:], in0=fprev[:], in1=fX[:], op=OP.subtract)
                if h == 0:
                    for g4 in range(0, NBLK, 4):
                        nbg = min(4, NBLK - g4)
                        gb = wpool.tile([NP, 4 * H], f32, tag="gb")
                        for i in range(nbg):
                            b = g4 + i
                            nc.vector.tensor_scalar(
                                out=gb[:, i * H : (i + 1) * H],
                                in0=f1[:, b * H : (b + 1) * H],
                                scalar1=dinvT[:, b : b + 1], scalar2=None, op0=OP.mult,
                            )
                        nc.scalar.dma_start(
                            out=gloc_v[1][:, g4 : g4 + nbg, :],
                            in_=gb[:, : nbg * H].rearrange("p (b d) -> p b d", b=nbg),
                        )
                    nc.sync.dma_start(out=gloc[1][NPAD : NPAD + 1, :], in_=zrow[:])
                    nc.gpsimd.collective_compute(
                        "AllGather", OP.bypass, replica_groups=rg,
                        ins=[gloc[1].ap().opt()], outs=[gfull[1].ap().opt()],
                    )

            # ---------- phase E: L3 + L4 column-major ----------
            f2 = fX
            for t in range(0, NBLK, 4):
                nbg = min(4, NBLK - t)
                w = nbg * NP
                ps3 = pspool.tile([NP, 512], f32, tag="pS", space="PSUM")
                for k, (fk, mk) in enumerate(zip((f0, f1, f2), ("M0t", "M1t", "M2t"))):
                    fkT = wpool.tile([H, 512], f32, tag="fkT")
                    for pr in range((nbg + 1) // 2):
                        nbp = min(2, nbg - 2 * pr)  # blocks in this pair
                        psT = pspool.tile([NP, NP], f32, tag="pA", space="PSUM")
                        nc.tensor.transpose(
                            out=psT[: nbp * H, :],
                            in_=fk[:, (t + 2 * pr) * H : (t + 2 * pr + nbp) * H],
                            identity=ident[:],
                        )
                        nc.vector.tensor_copy(
                            fkT[:, 2 * pr * NP : 2 * pr * NP + NP], psT[0:H, :NP]
                        )
                        if nbp > 1:
                            nc.vector.tensor_copy(
                                fkT[:, (2 * pr + 1) * NP : (2 * pr + 2) * NP],
                                psT[H : 2 * H, :NP],
                            )
                    nc.tensor.matmul(
                        out=ps3[:H, :w], lhsT=W[mk][:], rhs=fkT[:, :w],
                        start=(k == 0), stop=(k == 2),
                    )
                h3T = wpool.tile([H, 512], f32, tag="h3T")
                nc.vector.tensor_scalar(
                    out=h3T[:, :w], in0=ps3[:H, :w],
                    scalar1=W["b3c"][:, 0:1], scalar2=0.0, op0=OP.add, op1=OP.max,
                )
                psO = pspool.tile([NP, 512], f32, tag="pS", space="PSUM")
                nc.tensor.matmul(out=psO[:C, :w], lhsT=W["W4t"][:], rhs=h3T[:, :w], start=True, stop=True)
                oT = wpool.tile([C, 512], f32, tag="oT")
                nc.vector.tensor_scalar(
                    out=oT[:, :w], in0=psO[:C, :w],
                    scalar1=W["b4c"][:, 0:1], scalar2=None, op0=OP.add,
                )
                nc.sync.dma_start(out=out_ext[:, t * NP : t * NP + w], in_=oT[:, :w])

    nc.compile()
    return nc


def kernel(**inputs):
    import concourse.bass_utils as bass_utils

    in_feat = np.asarray(inputs["in_feat"], dtype=np.float32)
    src = np.asarray(inputs["src"]).astype(np.int64)
    dst = np.asarray(inputs["dst"]).astype(np.int64)

    (calls, call_col, chunk_cols, K, idx_inputs, ridx_inputs, xt_in, dinv_in) = (
        _host_prep(in_feat, src, dst)
    )
    weights = _weights(
        np.asarray(inputs["W1"]), np.asarray(inputs["b1"]),
        np.asarray(inputs["W2"]), np.asarray(inputs["b2"]),
        np.asarray(inputs["W3"]), np.asarray(inputs["b3"]),
        np.asarray(inputs["W4"]), np.asarray(inputs["b4"]),
    )

    nc = _build_program(calls, call_col, chunk_cols, K)

    in_maps = []
    for c in range(M):
        im = {"xt": xt_in[c], "dinvT": dinv_in[c], "ridx": ridx_inputs[c]}
        for c4 in range(NCHUNK):
            im[f"idx{c4}"] = idx_inputs[c][c4]
        im.update(weights)
        in_maps.append(im)

    trace = bool(int(os.environ.get("BWGNN_TRACE", "0")))
    res = bass_utils.run_bass_kernel_spmd(nc, in_maps, list(range(M)), trace=trace)
    global LAST_EXEC_NS
    LAST_EXEC_NS = res.exec_time_ns

    full = np.empty((N, C), dtype=np.float32)
    for c in range(M):
        r = res.results[c]["out"]  # [C, NPAD]
        full[c * NL : (c + 1) * NL] = r[:, :NL].T
    return full


# revision 18
# speedup vs baseline: 1.1147x; 1.1147x over previous
"""BWGNN (Beta-Wavelet GNN) forward on 8 Trainium2 NeuronCores.

Dense phases run column-major in fp32r (folded-bias stationaries, fused
bias+relu on the ACT engine, paired TensorE transposes only where layout
flips are unavoidable). Hop gathers are emitted chunk-major, round-robin on
the 4 SWDGE queues, with realign gathers skewed two chunks behind and the
hop epilogue fused per realign quarter-part; deep tile pools keep 6 gathers
in flight.

Nodes are partitioned across 8 cores (12500 each); dense linears are
data-parallel. Each polynomial hop: scale rows by d^-1/2, AllGather the scaled
table, bulk random gather of in-edge src rows with dma_gather (int16 indices
against 4 src-range chunks, one SWDGE queue per call round-robin), strided
vector reduce per 128-node block, realign gather, fused epilogue.

kernel(**inputs) takes FULL inputs and returns the FULL [N, 2] output.
"""
import os
import numpy as np

LAST_EXEC_NS = None

N = 100000
E = 1600000
IN = 128
H = 64
C = 2
THETAS = [[3.0, -3.0, 0.75], [0.0, 3.0, -1.5], [0.0, 0.0, 0.75]]

M = 8            # cores
NL = N // M      # 12500 nodes per core
NP = 128
NBLK = (NL + NP - 1) // NP   # 98
NPAD = NBLK * NP             # 12544
TRr = NPAD + 1               # per-rank table rows (zero row at NPAD)
NCHUNK = 4
CHROWS = 2 * TRr             # table rows per chunk (2 ranks) = 25002 < 32768
PADIDX = NPAD                # chunk-relative row of the first rank's zero row
MAX_IDX_CALL = 4096
MAX_NB = 16                  # max blocks per gather call (bounds reduce tile)
PB = [0, 24, 48, 73, 98]     # realign quarter-part block boundaries


def _wrap_idx(flat):
    """int16 flat gather list -> [128, len/16] SBUF wrap (16 partitions, x8)."""
    iw = len(flat) // 16
    w = flat.reshape(iw, 16).T
    return np.ascontiguousarray(np.tile(w, (8, 1)).astype(np.int16))


def _host_prep(in_feat, src, dst):
    deg = np.bincount(dst, minlength=N)
    dinv = (1.0 / np.sqrt(np.maximum(deg, 1))).astype(np.float32)

    core_of = dst // NL
    chunk_of = src // (2 * NL)
    idx16 = ((src // NL - 2 * chunk_of) * TRr + src % NL).astype(np.int32)

    key = core_of * NCHUNK + chunk_of
    order = np.argsort(key, kind="stable")
    bounds = np.searchsorted(key[order], np.arange(M * NCHUNK + 1))

    K = np.zeros((NCHUNK, NBLK), dtype=np.int64)
    groups = {}
    degc_all = np.zeros((M, NCHUNK, NPAD), dtype=np.int64)
    ords = np.empty((M, NCHUNK, NPAD), dtype=np.int64)
    lanes = np.empty((M, NCHUNK, NPAD), dtype=np.int32)
    for c in range(M):
        for c4 in range(NCHUNK):
            g = order[bounds[c * NCHUNK + c4] : bounds[c * NCHUNK + c4 + 1]]
            groups[(c, c4)] = g
            dl = dst[g] - c * NL
            dc = np.bincount(dl, minlength=NPAD)
            degc_all[c, c4] = dc
            o = np.argsort(-dc, kind="stable")
            ords[c, c4] = o
            inv = np.empty(NPAD, dtype=np.int32)
            inv[o] = np.arange(NPAD, dtype=np.int32)
            lanes[c, c4] = inv
            K[c4] = np.maximum(K[c4], dc[o].reshape(NBLK, NP)[:, 0])

    # call schedule: per chunk, runs of equal-K consecutive blocks, capped
    calls = []  # (c4, kb, b0, nb, nidx)
    for c4 in range(NCHUNK):
        b = 0
        while b < NBLK:
            kb = int(K[c4][b])
            if kb == 0:
                b += 1
                continue
            e_ = b
            while e_ + 1 < NBLK and int(K[c4][e_ + 1]) == kb:
                e_ += 1
            maxnb = min(MAX_NB, max(1, MAX_IDX_CALL // (NP * kb)))
            while b <= e_:
                nb = min(maxnb, e_ - b + 1)
                calls.append((c4, kb, b, nb, NP * kb * nb))
                b += nb

    chunk_cols = [0] * NCHUNK
    call_col = []
    for (c4, kb, b0, nb, nidx) in calls:
        call_col.append(chunk_cols[c4])
        chunk_cols[c4] += nidx // 16

    idx_inputs = []
    ridx_inputs = []
    for c in range(M):
        per_chunk = []
        for c4 in range(NCHUNK):
            g = groups[(c, c4)]
            dl = dst[g] - c * NL
            lane = lanes[c, c4][dl].astype(np.int64)
            eorder = np.argsort(lane, kind="stable")
            ge = g[eorder]
            lane_s = lane[eorder]
            counts = degc_all[c, c4][ords[c, c4]]
            starts = np.zeros(NPAD + 1, dtype=np.int64)
            np.cumsum(counts, out=starts[1:])
            slot = np.arange(len(ge)) - starts[lane_s]
            flat = np.full(chunk_cols[c4] * 16, PADIDX, dtype=np.int32)
            blk = lane_s // NP
            j = lane_s % NP
            for ci, (cc4, kb, b0, nb, nidx) in enumerate(calls):
                if cc4 != c4:
                    continue
                sel = (blk >= b0) & (blk < b0 + nb) & (slot < kb)
                base = call_col[ci] * 16
                pos = base + ((blk[sel] - b0) * kb + slot[sel]) * NP + j[sel]
                flat[pos] = idx16[ge[sel]]
            per_chunk.append(_wrap_idx(flat.astype(np.int16)))
        idx_inputs.append(per_chunk)
        rflat = np.concatenate(
            [lanes[c, c4][:NPAD].astype(np.int16) for c4 in range(NCHUNK)]
        )
        ridx_inputs.append(_wrap_idx(rflat))

    xt_in, dinv_in = [], []
    for c in range(M):
        xt = np.zeros((IN, NPAD), dtype=np.float32)
        xt[:, :NL] = in_feat[c * NL : (c + 1) * NL].T
        xt_in.append(np.ascontiguousarray(xt))
        dv = np.ones(NPAD, dtype=np.float32)
        dv[:NL] = dinv[c * NL : (c + 1) * NL]
        dinv_in.append(np.ascontiguousarray(dv.reshape(NBLK, NP).T))
    return calls, call_col, chunk_cols, K, idx_inputs, ridx_inputs, xt_in, dinv_in


def _weights(W1, b1, W2, b2, W3, b3, W4, b4):
    Mk = [
        sum(THETAS[t][k] * W3[:, t * H : (t + 1) * H] for t in range(len(THETAS)))
        for k in range(3)
    ]
    return {
        "W1t": np.ascontiguousarray(W1.T.astype(np.float32)),
        "W2t": np.ascontiguousarray(W2.T.astype(np.float32)),
        "M0t": np.ascontiguousarray(Mk[0].T.astype(np.float32)),
        "M1t": np.ascontiguousarray(Mk[1].T.astype(np.float32)),
        "M2t": np.ascontiguousarray(Mk[2].T.astype(np.float32)),
        "W4t": np.ascontiguousarray(W4.T.astype(np.float32)),
        "W2tb": np.ascontiguousarray(
            np.vstack([W2.T, b2.reshape(1, H)]).astype(np.float32)
        ),
        "W4tb": np.ascontiguousarray(
            np.vstack([W4.T, b4.reshape(1, C)]).astype(np.float32)
        ),
        "b1c": b1.reshape(H, 1).astype(np.float32),
        "b3c": b3.reshape(H, 1).astype(np.float32),
    }


def _build_program(calls, call_col, chunk_cols, K):
    import concourse.bacc as bacc
    import concourse.mybir as mybir
    import concourse.tile as tile
    from concourse.library_config import mlp
    from concourse.masks import make_identity

    f32 = mybir.dt.float32
    f32r = mybir.dt.float32r
    AF = mybir.ActivationFunctionType
    i16 = mybir.dt.int16
    AX = mybir.AxisListType
    OP = mybir.AluOpType

    nc = bacc.Bacc(
        "TRN2", target_bir_lowering=False, debug=False, num_devices=M,
        num_swdge_queues=4,
    )

    xt_ext = nc.declare_dram_parameter("xt", [IN, NPAD], f32r, isOutput=False)
    dinv_ext = nc.declare_dram_parameter("dinvT", [NP, NBLK], f32, isOutput=False)
    idx_ext = [
        nc.declare_dram_parameter(f"idx{c4}", [128, chunk_cols[c4]], i16, isOutput=False)
        for c4 in range(NCHUNK)
    ]
    ridx_ext = nc.declare_dram_parameter(
        "ridx", [128, NCHUNK * (NPAD // 16)], i16, isOutput=False
    )
    wshapes = [
        ("W1t", [IN, H], f32r), ("W2t", [H, H], f32r), ("M0t", [H, H], f32r),
        ("M1t", [H, H], f32r), ("M2t", [H, H], f32r), ("W4t", [H, C], f32r),
        ("W2tb", [H + 1, H], f32r), ("W4tb", [H + 1, C], f32r),
        ("b1c", [H, 1], f32), ("b3c", [H, 1], f32),
    ]
    wext = {nm: nc.declare_dram_parameter(nm, s, dt, isOutput=False) for nm, s, dt in wshapes}
    out_ext = nc.declare_dram_parameter("out", [C, NPAD], f32, isOutput=True)

    gloc = [nc.dram_tensor(f"g{h}loc", [TRr, H], f32) for h in range(2)]
    gfull = [
        nc.dram_tensor(f"g{h}full", [M * TRr, H], f32, addr_space="Shared")
        for h in range(2)
    ]
    aggdram = [nc.dram_tensor(f"agg{c4}", [NPAD, H], f32) for c4 in range(NCHUNK)]
    rg = [list(range(M))]

    with tile.TileContext(nc) as tc:
        with (
            tc.tile_pool(name="const", bufs=1) as cpool,
            tc.tile_pool(name="big", bufs=1) as bigpool,
            tc.tile_pool(name="xt", bufs=1) as xtpool,
            tc.tile_pool(name="work", bufs=2) as wpool,
            tc.tile_pool(name="idxp", bufs=2) as ipool,
            tc.tile_pool(name="gath", bufs=7) as gpool,
            tc.tile_pool(name="ridxp", bufs=2) as rpool,
            tc.tile_pool(name="ps", bufs=3, space="PSUM") as pspool,
        ):
            nc.gpsimd.load_library(mlp)

            W = {}
            for nm, s, dt in wshapes:
                W[nm] = cpool.tile(list(s), dt, tag=nm, name=nm)
                nc.sync.dma_start(out=W[nm][:], in_=wext[nm][:])
            dinvT = cpool.tile([NP, NBLK], f32, tag="dinvT")
            nc.sync.dma_start(out=dinvT[:], in_=dinv_ext[:])
            zrow = cpool.tile([1, H], f32, tag="zrow")
            nc.vector.memset(zrow[:], 0.0)
            zblk = cpool.tile([NP, H], f32, tag="zblk")
            nc.vector.memset(zblk[:], 0.0)
            ident = cpool.tile([NP, NP], f32, tag="ident")
            make_identity(nc, ident[:])
            ridx_t = cpool.tile([128, NCHUNK * (NPAD // 16)], i16, tag="ridx")
            nc.sync.dma_start(out=ridx_t[:], in_=ridx_ext[:])

            f0 = bigpool.tile([NP, NBLK * H], f32, tag="f0")
            f1 = bigpool.tile([NP, NBLK * H], f32, tag="f1")
            fX = bigpool.tile([NP, NBLK * H], f32, tag="fX")

            gloc_v = [g.ap()[0:NPAD, :].rearrange("(b j) d -> j b d", j=NP) for g in gloc]
            agg_v = [a.ap().rearrange("(b j) d -> j b d", j=NP) for a in aggdram]
            dbc = (
                dinvT[:]
                .rearrange("p (b o) -> p b o", o=1)
                .to_broadcast([NP, NBLK, H])
            )

            # ---------- phase A: L1 + L2 column-major, flip back per block ----
            GA = 4
            for g4 in range(0, NBLK, GA):
                nbg = min(GA, NBLK - g4)
                wg = nbg * NP
                xt = xtpool.tile([IN, GA * NP], f32r, tag="xt")
                nc.sync.dma_start(
                    out=xt[:, :wg], in_=xt_ext[:, g4 * NP : g4 * NP + wg]
                )
                ps1 = pspool.tile([NP, 512], f32, tag="pS", space="PSUM")
                nc.tensor.matmul(out=ps1[:H, :wg], lhsT=W["W1t"][:], rhs=xt[:, :wg], start=True, stop=True)
                h1X = wpool.tile([H + 1, GA * NP], f32r, tag="h1T")
                nc.scalar.activation(
                    h1X[:H, :wg], ps1[:H, :wg], AF.Relu, bias=W["b1c"][:, 0:1],
                )
                nc.vector.memset(h1X[H : H + 1, :wg].bitcast(f32), 1.0)
                gb = wpool.tile([NP, 25 * H], f32, tag="gbh")
                for i in range(nbg):
                    b = g4 + i
                    ps2b = pspool.tile([NP, NP], f32, tag="pA", space="PSUM")
                    nc.tensor.matmul(
                        out=ps2b[:, :H], lhsT=h1X[:, i * NP : (i + 1) * NP],
                        rhs=W["W2tb"][:], start=True, stop=True,
                    )
                    nc.vector.tensor_scalar(
                        out=f0[:, b * H : (b + 1) * H], in0=ps2b[:, :H],
                        scalar1=0.0, scalar2=None, op0=OP.max,
                    )
                dbc_s = (
                    dinvT[:, g4 : g4 + nbg]
                    .rearrange("p (b o) -> p b o", o=1)
                    .to_broadcast([NP, nbg, H])
                )
                nc.vector.tensor_tensor(
                    out=gb[:, : nbg * H].rearrange("p (b d) -> p b d", b=nbg),
                    in0=f0[:, g4 * H : (g4 + nbg) * H].rearrange("p (b d) -> p b d", b=nbg),
                    in1=dbc_s, op=OP.mult,
                )
                nc.scalar.dma_start(
                    out=gloc_v[0][:, g4 : g4 + nbg, :],
                    in_=gb[:, : nbg * H].rearrange("p (b d) -> p b d", b=nbg),
                )
            nc.sync.dma_start(out=gloc[0][NPAD : NPAD + 1, :], in_=zrow[:])

            nc.gpsimd.collective_compute(
                "AllGather", OP.bypass, replica_groups=rg,
                ins=[gloc[0].ap().opt()], outs=[gfull[0].ap().opt()],
            )

            MAXCC = max(chunk_cols)
            by_chunk = [[] for _ in range(NCHUNK)]
            for ci, cl in enumerate(calls):
                by_chunk[cl[0]].append((ci, cl))

            # ---------- two hops ----------
            for h in range(2):
                fprev = f0 if h == 0 else f1
                fout = f1 if h == 0 else fX
                table = gfull[h]

                def emit_realign(c4):
                    for p in range(4):
                        pb0, pb1 = PB[p], PB[p + 1]
                        npb = pb1 - pb0
                        rt = rpool.tile([NP, 25, H], f32, tag="rt")
                        coff = c4 * (NPAD // 16) + pb0 * (NP // 16)
                        nc.gpsimd.dma_gather(
                            rt[:, :npb, :],
                            aggdram[c4][:, :],
                            ridx_t[:, coff : coff + npb * (NP // 16)],
                            npb * NP,
                            npb * NP,
                            H,
                            single_packet=False,
                            queue_num=p,
                        )
                        rtf = rt[:, :npb, :].rearrange("p b d -> p (b d)")
                        fxs = fX[:, pb0 * H : pb1 * H]
                        if c4 == 0:
                            nc.vector.tensor_copy(fxs, rtf)
                        else:
                            nc.vector.tensor_tensor(out=fxs, in0=fxs, in1=rtf, op=OP.add)
                        if c4 == NCHUNK - 1:
                            # fused epilogue for this block range
                            dbc_p = (
                                dinvT[:, pb0:pb1]
                                .rearrange("p (b o) -> p b o", o=1)
                                .to_broadcast([NP, npb, H])
                            )
                            fx3 = fxs.rearrange("p (b d) -> p b d", b=npb)
                            nc.vector.tensor_tensor(out=fx3, in0=fx3, in1=dbc_p, op=OP.mult)
                            fo = fout[:, pb0 * H : pb1 * H]
                            nc.vector.tensor_tensor(
                                out=fo, in0=fprev[:, pb0 * H : pb1 * H], in1=fxs,
                                op=OP.subtract,
                            )
                            if h == 0:
                                gb = wpool.tile([NP, 25 * H], f32, tag="gbh")
                                nc.vector.tensor_tensor(
                                    out=gb[:, : npb * H].rearrange("p (b d) -> p b d", b=npb),
                                    in0=fo.rearrange("p (b d) -> p b d", b=npb),
                                    in1=dbc_p, op=OP.mult,
                                )
                                nc.scalar.dma_start(
                                    out=gloc_v[1][:, pb0:pb1, :],
                                    in_=gb[:, : npb * H].rearrange("p (b d) -> p b d", b=npb),
                                )

                qrr = 0
                for c4 in range(NCHUNK):
                    it = ipool.tile([128, MAXCC], i16, tag="idxc")
                    nc.sync.dma_start(
                        out=it[:, : chunk_cols[c4]], in_=idx_ext[c4][:]
                    )
                    for ci, (cc4, kb, b0, nb, nidx) in by_chunk[c4]:
                        S = nidx // NP
                        dst_t = gpool.tile([NP, MAX_IDX_CALL // NP, H], f32, tag="gdst")
                        nc.gpsimd.dma_gather(
                            dst_t[:, :S, :],
                            table[c4 * CHROWS : (c4 + 1) * CHROWS, :],
                            it[:, call_col[ci] : call_col[ci] + nidx // 16],
                            nidx,
                            nidx,
                            H,
                            single_packet=False,
                            queue_num=qrr % 4,
                        )
                        qrr += 1
                        red = wpool.tile([NP, MAX_NB * H], f32, tag="red")
                        nc.vector.tensor_reduce(
                            out=red[:, : nb * H].rearrange("p (b d) -> p b d", b=nb),
                            in_=dst_t[:, :S, :].rearrange("p (b k) d -> p b d k", b=nb, k=kb),
                            axis=AX.X,
                            op=OP.add,
                        )
                        nc.scalar.dma_start(
                            out=agg_v[c4][:, b0 : b0 + nb, :],
                            in_=red[:, : nb * H].rearrange("p (b d) -> p b d", b=nb),
                        )
                    for b in range(NBLK):
                        if K[c4][b] == 0:
                            nc.sync.dma_start(
                                out=aggdram[c4][b * NP : (b + 1) * NP, :], in_=zblk[:]
                            )
                    if c4 >= 2:
                        emit_realign(c4 - 2)
                emit_realign(NCHUNK - 2)
                emit_realign(NCHUNK - 1)

                if h == 0:
                    nc.sync.dma_start(out=gloc[1][NPAD : NPAD + 1, :], in_=zrow[:])
                    nc.gpsimd.collective_compute(
                        "AllGather", OP.bypass, replica_groups=rg,
                        ins=[gloc[1].ap().opt()], outs=[gfull[1].ap().opt()],
                    )

            # ---------- phase E: L3 + L4 column-major ----------
            f2 = fX
            for t in range(0, NBLK, 4):
                nbg = min(4, NBLK - t)
                w = nbg * NP
                ps3 = pspool.tile([NP, 512], f32, tag="pS", space="PSUM")
                for k, (fk, mk) in enumerate(zip((f0, f1, f2), ("M0t", "M1t", "M2t"))):
                    fkT = wpool.tile([H, 512], f32r, tag="fkT")
                    for pr in range((nbg + 1) // 2):
                        nbp = min(2, nbg - 2 * pr)  # blocks in this pair
                        psT = pspool.tile([NP, NP], f32, tag="pA", space="PSUM")
                        nc.tensor.transpose(
                            out=psT[: nbp * H, :],
                            in_=fk[:, (t + 2 * pr) * H : (t + 2 * pr + nbp) * H],
                            identity=ident[:],
                        )
                        nc.vector.tensor_copy(
                            fkT[:, 2 * pr * NP : 2 * pr * NP + NP], psT[0:H, :NP]
                        )
                        if nbp > 1:
                            nc.scalar.copy(
                                fkT[:, (2 * pr + 1) * NP : (2 * pr + 2) * NP],
                                psT[H : 2 * H, :NP],
                            )
                    nc.tensor.matmul(
                        out=ps3[:H, :w], lhsT=W[mk][:], rhs=fkT[:, :w],
                        start=(k == 0), stop=(k == 2),
                    )
                h3X = wpool.tile([H + 1, 512], f32r, tag="h3T")
                nc.scalar.activation(
                    h3X[:H, :w], ps3[:H, :w], AF.Relu, bias=W["b3c"][:, 0:1],
                )
                nc.vector.memset(h3X[H : H + 1, :w].bitcast(f32), 1.0)
                psO = pspool.tile([NP, 512], f32, tag="pS", space="PSUM")
                nc.tensor.matmul(out=psO[:C, :w], lhsT=W["W4tb"][:], rhs=h3X[:, :w], start=True, stop=True)
                oT = wpool.tile([C, 512], f32, tag="oT")
                nc.vector.tensor_copy(oT[:, :w], psO[:C, :w])
                nc.sync.dma_start(out=out_ext[:, t * NP : t * NP + w], in_=oT[:, :w])

    nc.compile()
    return nc


def kernel(**inputs):
    import concourse.bass_utils as bass_utils

    in_feat = np.asarray(inputs["in_feat"], dtype=np.float32)
    src = np.asarray(inputs["src"]).astype(np.int64)
    dst = np.asarray(inputs["dst"]).astype(np.int64)

    (calls, call_col, chunk_cols, K, idx_inputs, ridx_inputs, xt_in, dinv_in) = (
        _host_prep(in_feat, src, dst)
    )
    weights = _weights(
        np.asarray(inputs["W1"]), np.asarray(inputs["b1"]),
        np.asarray(inputs["W2"]), np.asarray(inputs["b2"]),
        np.asarray(inputs["W3"]), np.asarray(inputs["b3"]),
        np.asarray(inputs["W4"]), np.asarray(inputs["b4"]),
    )

    nc = _build_program(calls, call_col, chunk_cols, K)

    in_maps = []
    for c in range(M):
        im = {"xt": xt_in[c], "dinvT": dinv_in[c], "ridx": ridx_inputs[c]}
        for c4 in range(NCHUNK):
            im[f"idx{c4}"] = idx_inputs[c][c4]
        im.update(weights)
        in_maps.append(im)

    trace = bool(int(os.environ.get("BWGNN_TRACE", "0")))
    res = bass_utils.run_bass_kernel_spmd(nc, in_maps, list(range(M)), trace=trace)
    global LAST_EXEC_NS
    LAST_EXEC_NS = res.exec_time_ns

    full = np.empty((N, C), dtype=np.float32)
    for c in range(M):
        r = res.results[c]["out"]  # [C, NPAD]
        full[c * NL : (c + 1) * NL] = r[:, :NL].T
    return full


# revision 20
# speedup vs baseline: 1.1575x; 1.0383x over previous
"""BWGNN (Beta-Wavelet GNN) forward on 8 Trainium2 NeuronCores.

Dense phases run column-major in fp32r (folded-bias stationaries, fused
bias+relu on the ACT engine, paired TensorE transposes only where layout
flips are unavoidable). Hop gathers are emitted chunk-major, round-robin on
the 4 SWDGE queues, with realign gathers skewed two chunks behind and the
hop epilogue fused per realign quarter-part; deep tile pools keep 6 gathers
in flight.

Nodes are partitioned across 8 cores (12500 each); dense linears are
data-parallel. Each polynomial hop: scale rows by d^-1/2, AllGather the scaled
table, bulk random gather of in-edge src rows with dma_gather (int16 indices
against 4 src-range chunks, one SWDGE queue per call round-robin), strided
vector reduce per 128-node block, realign gather, fused epilogue.

kernel(**inputs) takes FULL inputs and returns the FULL [N, 2] output.
"""
import os
import numpy as np

LAST_EXEC_NS = None

N = 100000
E = 1600000
IN = 128
H = 64
C = 2
THETAS = [[3.0, -3.0, 0.75], [0.0, 3.0, -1.5], [0.0, 0.0, 0.75]]

M = 8            # cores
NL = N // M      # 12500 nodes per core
NP = 128
NBLK = (NL + NP - 1) // NP   # 98
NPAD = NBLK * NP             # 12544
TRr = NPAD + 1               # per-rank table rows (zero row at NPAD)
NCHUNK = 4
CHROWS = 2 * TRr             # table rows per chunk (2 ranks) = 25002 < 32768
PADIDX = NPAD                # chunk-relative row of the first rank's zero row
MAX_IDX_CALL = 4096
MAX_NB = 16                  # max blocks per gather call (bounds reduce tile)
PB = [0, 24, 48, 73, 98]     # realign quarter-part block boundaries


def _wrap_idx(flat):
    """int16 flat gather list -> [128, len/16] SBUF wrap (16 partitions, x8)."""
    iw = len(flat) // 16
    w = flat.reshape(iw, 16).T
    return np.ascontiguousarray(np.tile(w, (8, 1)).astype(np.int16))


def _host_prep(in_feat, src, dst):
    deg = np.bincount(dst, minlength=N)
    dinv = (1.0 / np.sqrt(np.maximum(deg, 1))).astype(np.float32)

    core_of = dst // NL
    chunk_of = src // (2 * NL)
    idx16 = ((src // NL - 2 * chunk_of) * TRr + src % NL).astype(np.int32)

    key = core_of * NCHUNK + chunk_of
    order = np.argsort(key, kind="stable")
    bounds = np.searchsorted(key[order], np.arange(M * NCHUNK + 1))

    K = np.zeros((NCHUNK, NBLK), dtype=np.int64)
    groups = {}
    degc_all = np.zeros((M, NCHUNK, NPAD), dtype=np.int64)
    ords = np.empty((M, NCHUNK, NPAD), dtype=np.int64)
    lanes = np.empty((M, NCHUNK, NPAD), dtype=np.int32)
    for c in range(M):
        for c4 in range(NCHUNK):
            g = order[bounds[c * NCHUNK + c4] : bounds[c * NCHUNK + c4 + 1]]
            groups[(c, c4)] = g
            dl = dst[g] - c * NL
            dc = np.bincount(dl, minlength=NPAD)
            degc_all[c, c4] = dc
            o = np.argsort(-dc, kind="stable")
            ords[c, c4] = o
            inv = np.empty(NPAD, dtype=np.int32)
            inv[o] = np.arange(NPAD, dtype=np.int32)
            lanes[c, c4] = inv
            K[c4] = np.maximum(K[c4], dc[o].reshape(NBLK, NP)[:, 0])

    # call schedule: per chunk, runs of equal-K consecutive blocks, capped
    calls = []  # (c4, kb, b0, nb, nidx)
    for c4 in range(NCHUNK):
        b = 0
        while b < NBLK:
            kb = int(K[c4][b])
            if kb == 0:
                b += 1
                continue
            e_ = b
            while e_ + 1 < NBLK and int(K[c4][e_ + 1]) == kb:
                e_ += 1
            maxnb = min(MAX_NB, max(1, MAX_IDX_CALL // (NP * kb)))
            while b <= e_:
                nb = min(maxnb, e_ - b + 1)
                calls.append((c4, kb, b, nb, NP * kb * nb))
                b += nb

    chunk_cols = [0] * NCHUNK
    call_col = []
    for (c4, kb, b0, nb, nidx) in calls:
        call_col.append(chunk_cols[c4])
        chunk_cols[c4] += nidx // 16

    idx_inputs = []
    ridx_inputs = []
    for c in range(M):
        per_chunk = []
        for c4 in range(NCHUNK):
            g = groups[(c, c4)]
            dl = dst[g] - c * NL
            lane = lanes[c, c4][dl].astype(np.int64)
            eorder = np.argsort(lane, kind="stable")
            ge = g[eorder]
            lane_s = lane[eorder]
            counts = degc_all[c, c4][ords[c, c4]]
            starts = np.zeros(NPAD + 1, dtype=np.int64)
            np.cumsum(counts, out=starts[1:])
            slot = np.arange(len(ge)) - starts[lane_s]
            flat = np.full(chunk_cols[c4] * 16, PADIDX, dtype=np.int32)
            blk = lane_s // NP
            j = lane_s % NP
            for ci, (cc4, kb, b0, nb, nidx) in enumerate(calls):
                if cc4 != c4:
                    continue
                sel = (blk >= b0) & (blk < b0 + nb) & (slot < kb)
                base = call_col[ci] * 16
                pos = base + ((blk[sel] - b0) * kb + slot[sel]) * NP + j[sel]
                flat[pos] = idx16[ge[sel]]
            per_chunk.append(_wrap_idx(flat.astype(np.int16)))
        idx_inputs.append(per_chunk)
        rflat = np.concatenate(
            [lanes[c, c4][:NPAD].astype(np.int16) for c4 in range(NCHUNK)]
        )
        ridx_inputs.append(_wrap_idx(rflat))

    xt_in, dinv_in = [], []
    for c in range(M):
        xt = np.zeros((IN, NPAD), dtype=np.float32)
        xt[:, :NL] = in_feat[c * NL : (c + 1) * NL].T
        xt_in.append(np.ascontiguousarray(xt))
        dv = np.ones(NPAD, dtype=np.float32)
        dv[:NL] = dinv[c * NL : (c + 1) * NL]
        dinv_in.append(np.ascontiguousarray(dv.reshape(NBLK, NP).T))
    return calls, call_col, chunk_cols, K, idx_inputs, ridx_inputs, xt_in, dinv_in


def _weights(W1, b1, W2, b2, W3, b3, W4, b4):
    Mk = [
        sum(THETAS[t][k] * W3[:, t * H : (t + 1) * H] for t in range(len(THETAS)))
        for k in range(3)
    ]
    return {
        "W1t": np.ascontiguousarray(W1.T.astype(np.float32)),
        "W2t": np.ascontiguousarray(W2.T.astype(np.float32)),
        "M0t": np.ascontiguousarray(Mk[0].T.astype(np.float32)),
        "M1t": np.ascontiguousarray(Mk[1].T.astype(np.float32)),
        "M2t": np.ascontiguousarray(Mk[2].T.astype(np.float32)),
        "W4t": np.ascontiguousarray(W4.T.astype(np.float32)),
        "W2tb": np.ascontiguousarray(
            np.vstack([W2.T, b2.reshape(1, H)]).astype(np.float32)
        ),
        "W4tb": np.ascontiguousarray(
            np.vstack([W4.T, b4.reshape(1, C)]).astype(np.float32)
        ),
        "b1c": b1.reshape(H, 1).astype(np.float32),
        "b3c": b3.reshape(H, 1).astype(np.float32),
    }


def _build_program(calls, call_col, chunk_cols, K):
    import concourse.bacc as bacc
    import concourse.mybir as mybir
    import concourse.tile as tile
    from concourse.library_config import mlp
    from concourse.masks import make_identity

    f32 = mybir.dt.float32
    f32r = mybir.dt.float32r
    AF = mybir.ActivationFunctionType
    i16 = mybir.dt.int16
    AX = mybir.AxisListType
    OP = mybir.AluOpType

    nc = bacc.Bacc(
        "TRN2", target_bir_lowering=False, debug=False, num_devices=M,
        num_swdge_queues=4,
    )

    xt_ext = nc.declare_dram_parameter("xt", [IN, NPAD], f32r, isOutput=False)
    dinv_ext = nc.declare_dram_parameter("dinvT", [NP, NBLK], f32, isOutput=False)
    idx_ext = [
        nc.declare_dram_parameter(f"idx{c4}", [128, chunk_cols[c4]], i16, isOutput=False)
        for c4 in range(NCHUNK)
    ]
    ridx_ext = nc.declare_dram_parameter(
        "ridx", [128, NCHUNK * (NPAD // 16)], i16, isOutput=False
    )
    wshapes = [
        ("W1t", [IN, H], f32r), ("W2t", [H, H], f32r), ("M0t", [H, H], f32r),
        ("M1t", [H, H], f32r), ("M2t", [H, H], f32r), ("W4t", [H, C], f32r),
        ("W2tb", [H + 1, H], f32r), ("W4tb", [H + 1, C], f32r),
        ("b1c", [H, 1], f32), ("b3c", [H, 1], f32),
    ]
    wext = {nm: nc.declare_dram_parameter(nm, s, dt, isOutput=False) for nm, s, dt in wshapes}
    out_ext = nc.declare_dram_parameter("out", [C, NPAD], f32, isOutput=True)

    gloc = [nc.dram_tensor(f"g{h}loc", [TRr, H], f32) for h in range(2)]
    gfull = [
        nc.dram_tensor(f"g{h}full", [M * TRr, H], f32, addr_space="Shared")
        for h in range(2)
    ]
    aggdram = [nc.dram_tensor(f"agg{c4}", [NPAD, H], f32) for c4 in range(NCHUNK)]
    rg = [list(range(M))]

    with tile.TileContext(nc) as tc:
        with (
            tc.tile_pool(name="const", bufs=1) as cpool,
            tc.tile_pool(name="big", bufs=1) as bigpool,
            tc.tile_pool(name="xt", bufs=2) as xtpool,
            tc.tile_pool(name="work", bufs=2) as wpool,
            tc.tile_pool(name="idxp", bufs=2) as ipool,
            tc.tile_pool(name="gath", bufs=6) as gpool,
            tc.tile_pool(name="ridxp", bufs=3) as rpool,
            tc.tile_pool(name="ps", bufs=4, space="PSUM") as pspool,
        ):
            nc.gpsimd.load_library(mlp)

            W = {}
            for nm, s, dt in wshapes:
                W[nm] = cpool.tile(list(s), dt, tag=nm, name=nm)
                nc.sync.dma_start(out=W[nm][:], in_=wext[nm][:])
            dinvT = cpool.tile([NP, NBLK], f32, tag="dinvT")
            nc.sync.dma_start(out=dinvT[:], in_=dinv_ext[:])
            zrow = cpool.tile([1, H], f32, tag="zrow")
            nc.vector.memset(zrow[:], 0.0)
            zblk = cpool.tile([NP, H], f32, tag="zblk")
            nc.vector.memset(zblk[:], 0.0)
            ident = cpool.tile([NP, NP], f32, tag="ident")
            make_identity(nc, ident[:])
            ridx_t = cpool.tile([128, NCHUNK * (NPAD // 16)], i16, tag="ridx")
            nc.sync.dma_start(out=ridx_t[:], in_=ridx_ext[:])

            f0 = bigpool.tile([NP, NBLK * H], f32, tag="f0")
            f1 = bigpool.tile([NP, NBLK * H], f32, tag="f1")
            fX = bigpool.tile([NP, NBLK * H], f32, tag="fX")

            gloc_v = [g.ap()[0:NPAD, :].rearrange("(b j) d -> j b d", j=NP) for g in gloc]
            agg_v = [a.ap().rearrange("(b j) d -> j b d", j=NP) for a in aggdram]
            dbc = (
                dinvT[:]
                .rearrange("p (b o) -> p b o", o=1)
                .to_broadcast([NP, NBLK, H])
            )

            # ---------- phase A: L1 + L2 column-major, flip back per block ----
            GA = 4
            for g4 in range(0, NBLK, GA):
                nbg = min(GA, NBLK - g4)
                wg = nbg * NP
                xt = xtpool.tile([IN, GA * NP], f32r, tag="xt")
                nc.sync.dma_start(
                    out=xt[:, :wg], in_=xt_ext[:, g4 * NP : g4 * NP + wg]
                )
                ps1 = pspool.tile([NP, 512], f32, tag="pS", space="PSUM")
                nc.tensor.matmul(out=ps1[:H, :wg], lhsT=W["W1t"][:], rhs=xt[:, :wg], start=True, stop=True)
                h1X = wpool.tile([H + 1, GA * NP], f32r, tag="h1T")
                nc.scalar.activation(
                    h1X[:H, :wg], ps1[:H, :wg], AF.Relu, bias=W["b1c"][:, 0:1],
                )
                nc.vector.memset(h1X[H : H + 1, :wg].bitcast(f32), 1.0)
                gb = wpool.tile([NP, 25 * H], f32, tag="gbh")
                for i in range(nbg):
                    b = g4 + i
                    ps2b = pspool.tile([NP, NP], f32, tag="pA", space="PSUM")
                    nc.tensor.matmul(
                        out=ps2b[:, :H], lhsT=h1X[:, i * NP : (i + 1) * NP],
                        rhs=W["W2tb"][:], start=True, stop=True,
                    )
                    nc.vector.tensor_scalar(
                        out=f0[:, b * H : (b + 1) * H], in0=ps2b[:, :H],
                        scalar1=0.0, scalar2=None, op0=OP.max,
                    )
                dbc_s = (
                    dinvT[:, g4 : g4 + nbg]
                    .rearrange("p (b o) -> p b o", o=1)
                    .to_broadcast([NP, nbg, H])
                )
                nc.vector.tensor_tensor(
                    out=gb[:, : nbg * H].rearrange("p (b d) -> p b d", b=nbg),
                    in0=f0[:, g4 * H : (g4 + nbg) * H].rearrange("p (b d) -> p b d", b=nbg),
                    in1=dbc_s, op=OP.mult,
                )
                nc.scalar.dma_start(
                    out=gloc_v[0][:, g4 : g4 + nbg, :],
                    in_=gb[:, : nbg * H].rearrange("p (b d) -> p b d", b=nbg),
                )
            nc.sync.dma_start(out=gloc[0][NPAD : NPAD + 1, :], in_=zrow[:])

            nc.gpsimd.collective_compute(
                "AllGather", OP.bypass, replica_groups=rg,
                ins=[gloc[0].ap().opt()], outs=[gfull[0].ap().opt()],
            )

            MAXCC = max(chunk_cols)
            by_chunk = [[] for _ in range(NCHUNK)]
            for ci, cl in enumerate(calls):
                by_chunk[cl[0]].append((ci, cl))

            # ---------- two hops ----------
            for h in range(2):
                fprev = f0 if h == 0 else f1
                fout = f1 if h == 0 else fX
                table = gfull[h]

                def emit_realign(c4):
                    for p in range(4):
                        pb0, pb1 = PB[p], PB[p + 1]
                        npb = pb1 - pb0
                        rt = rpool.tile([NP, 25, H], f32, tag="rt")
                        coff = c4 * (NPAD // 16) + pb0 * (NP // 16)
                        nc.gpsimd.dma_gather(
                            rt[:, :npb, :],
                            aggdram[c4][:, :],
                            ridx_t[:, coff : coff + npb * (NP // 16)],
                            npb * NP,
                            npb * NP,
                            H,
                            single_packet=False,
                            queue_num=p,
                        )
                        rtf = rt[:, :npb, :].rearrange("p b d -> p (b d)")
                        fxs = fX[:, pb0 * H : pb1 * H]
                        if c4 == 0:
                            nc.vector.tensor_copy(fxs, rtf)
                        else:
                            nc.vector.tensor_tensor(out=fxs, in0=fxs, in1=rtf, op=OP.add)
                        if c4 == NCHUNK - 1:
                            # fused epilogue for this block range
                            dbc_p = (
                                dinvT[:, pb0:pb1]
                                .rearrange("p (b o) -> p b o", o=1)
                                .to_broadcast([NP, npb, H])
                            )
                            fx3 = fxs.rearrange("p (b d) -> p b d", b=npb)
                            nc.vector.tensor_tensor(out=fx3, in0=fx3, in1=dbc_p, op=OP.mult)
                            fo = fout[:, pb0 * H : pb1 * H]
                            nc.vector.tensor_tensor(
                                out=fo, in0=fprev[:, pb0 * H : pb1 * H], in1=fxs,
                                op=OP.subtract,
                            )
                            if h == 0:
                                gb = wpool.tile([NP, 25 * H], f32, tag="gbh")
                                nc.vector.tensor_tensor(
                                    out=gb[:, : npb * H].rearrange("p (b d) -> p b d", b=npb),
                                    in0=fo.rearrange("p (b d) -> p b d", b=npb),
                                    in1=dbc_p, op=OP.mult,
                                )
                                nc.scalar.dma_start(
                                    out=gloc_v[1][:, pb0:pb1, :],
                                    in_=gb[:, : npb * H].rearrange("p (b d) -> p b d", b=npb),
                                )

                qrr = 0
                for c4 in range(NCHUNK):
                    it = ipool.tile([128, MAXCC], i16, tag="idxc")
                    nc.sync.dma_start(
                        out=it[:, : chunk_cols[c4]], in_=idx_ext[c4][:]
                    )
                    for ci, (cc4, kb, b0, nb, nidx) in by_chunk[c4]:
                        S = nidx // NP
                        dst_t = gpool.tile([NP, MAX_IDX_CALL // NP, H], f32, tag="gdst")
                        nc.gpsimd.dma_gather(
                            dst_t[:, :S, :],
                            table[c4 * CHROWS : (c4 + 1) * CHROWS, :],
                            it[:, call_col[ci] : call_col[ci] + nidx // 16],
                            nidx,
                            nidx,
                            H,
                            single_packet=False,
                            queue_num=qrr % 4,
                        )
                        qrr += 1
                        red = wpool.tile([NP, MAX_NB * H], f32, tag="red")
                        nc.vector.tensor_reduce(
                            out=red[:, : nb * H].rearrange("p (b d) -> p b d", b=nb),
                            in_=dst_t[:, :S, :].rearrange("p (b k) d -> p b d k", b=nb, k=kb),
                            axis=AX.X,
                            op=OP.add,
                        )
                        nc.scalar.dma_start(
                            out=agg_v[c4][:, b0 : b0 + nb, :],
                            in_=red[:, : nb * H].rearrange("p (b d) -> p b d", b=nb),
                        )
                    for b in range(NBLK):
                        if K[c4][b] == 0:
                            nc.sync.dma_start(
                                out=aggdram[c4][b * NP : (b + 1) * NP, :], in_=zblk[:]
                            )
                    if c4 >= 2:
                        emit_realign(c4 - 2)
                emit_realign(NCHUNK - 2)
                emit_realign(NCHUNK - 1)

                if h == 0:
                    nc.sync.dma_start(out=gloc[1][NPAD : NPAD + 1, :], in_=zrow[:])
                    nc.gpsimd.collective_compute(
                        "AllGather", OP.bypass, replica_groups=rg,
                        ins=[gloc[1].ap().opt()], outs=[gfull[1].ap().opt()],
                    )

            # ---------- phase E: L3 + L4 column-major ----------
            f2 = fX
            for t in range(0, NBLK, 4):
                nbg = min(4, NBLK - t)
                w = nbg * NP
                ps3 = pspool.tile([NP, 512], f32, tag="pS", space="PSUM")
                for k, (fk, mk) in enumerate(zip((f0, f1, f2), ("M0t", "M1t", "M2t"))):
                    fkT = wpool.tile([H, 512], f32r, tag="fkT")
                    for pr in range((nbg + 1) // 2):
                        nbp = min(2, nbg - 2 * pr)  # blocks in this pair
                        psT = pspool.tile([NP, NP], f32, tag="pA", space="PSUM")
                        nc.tensor.transpose(
                            out=psT[: nbp * H, :],
                            in_=fk[:, (t + 2 * pr) * H : (t + 2 * pr + nbp) * H],
                            identity=ident[:],
                        )
                        nc.vector.tensor_copy(
                            fkT[:, 2 * pr * NP : 2 * pr * NP + NP], psT[0:H, :NP]
                        )
                        if nbp > 1:
                            nc.scalar.copy(
                                fkT[:, (2 * pr + 1) * NP : (2 * pr + 2) * NP],
                                psT[H : 2 * H, :NP],
                            )
                    nc.tensor.matmul(
                        out=ps3[:H, :w], lhsT=W[mk][:], rhs=fkT[:, :w],
                        start=(k == 0), stop=(k == 2),
                    )
                h3X = wpool.tile([H + 1, 512], f32r, tag="h3T")
                nc.scalar.activation(
                    h3X[:H, :w], ps3[:H, :w], AF.Relu, bias=W["b3c"][:, 0:1],
                )
                nc.vector.memset(h3X[H : H + 1, :w].bitcast(f32), 1.0)
                psO = pspool.tile([NP, 512], f32, tag="pS", space="PSUM")
                nc.tensor.matmul(out=psO[:C, :w], lhsT=W["W4tb"][:], rhs=h3X[:, :w], start=True, stop=True)
                oT = wpool.tile([C, 512], f32, tag="oT")
                nc.vector.tensor_copy(oT[:, :w], psO[:C, :w])
                nc.sync.dma_start(out=out_ext[:, t * NP : t * NP + w], in_=oT[:, :w])

    nc.compile()
    return nc


def kernel(**inputs):
    import concourse.bass_utils as bass_utils

    in_feat = np.asarray(inputs["in_feat"], dtype=np.float32)
    src = np.asarray(inputs["src"]).astype(np.int64)
    dst = np.asarray(inputs["dst"]).astype(np.int64)

    (calls, call_col, chunk_cols, K, idx_inputs, ridx_inputs, xt_in, dinv_in) = (
        _host_prep(in_feat, src, dst)
    )
    weights = _weights(
        np.asarray(inputs["W1"]), np.asarray(inputs["b1"]),
        np.asarray(inputs["W2"]), np.asarray(inputs["b2"]),
        np.asarray(inputs["W3"]), np.asarray(inputs["b3"]),
        np.asarray(inputs["W4"]), np.asarray(inputs["b4"]),
    )

    nc = _build_program(calls, call_col, chunk_cols, K)

    in_maps = []
    for c in range(M):
        im = {"xt": xt_in[c], "dinvT": dinv_in[c], "ridx": ridx_inputs[c]}
        for c4 in range(NCHUNK):
            im[f"idx{c4}"] = idx_inputs[c][c4]
        im.update(weights)
        in_maps.append(im)

    trace = bool(int(os.environ.get("BWGNN_TRACE", "0")))
    res = bass_utils.run_bass_kernel_spmd(nc, in_maps, list(range(M)), trace=trace)
    global LAST_EXEC_NS
    LAST_EXEC_NS = res.exec_time_ns

    full = np.empty((N, C), dtype=np.float32)
    for c in range(M):
        r = res.results[c]["out"]  # [C, NPAD]
        full[c * NL : (c + 1) * NL] = r[:, :NL].T
    return full


# revision 22
# speedup vs baseline: 1.1649x; 1.0065x over previous
"""BWGNN (Beta-Wavelet GNN) forward on 8 Trainium2 NeuronCores.

Dense phases run column-major in fp32r (folded-bias stationaries, fused
bias+relu on the ACT engine, paired TensorE transposes only where layout
flips are unavoidable). Hop gathers are emitted chunk-major, round-robin on
the 4 SWDGE queues, with realign gathers skewed two chunks behind and the
hop epilogue fused per realign quarter-part; deep tile pools keep 6 gathers
in flight.

Nodes are partitioned across 8 cores (12500 each); dense linears are
data-parallel. Each polynomial hop: scale rows by d^-1/2, AllGather the scaled
table, bulk random gather of in-edge src rows with dma_gather (int16 indices
against 4 src-range chunks, one SWDGE queue per call round-robin), strided
vector reduce per 128-node block, realign gather, fused epilogue.

kernel(**inputs) takes FULL inputs and returns the FULL [N, 2] output.
"""
import os
import numpy as np

LAST_EXEC_NS = None

N = 100000
E = 1600000
IN = 128
H = 64
C = 2
THETAS = [[3.0, -3.0, 0.75], [0.0, 3.0, -1.5], [0.0, 0.0, 0.75]]

M = 8            # cores
NL = N // M      # 12500 nodes per core
NP = 128
NBLK = (NL + NP - 1) // NP   # 98
NPAD = NBLK * NP             # 12544
TRr = NPAD + 1               # per-rank table rows (zero row at NPAD)
NCHUNK = 4
CHROWS = 2 * TRr             # table rows per chunk (2 ranks) = 25002 < 32768
PADIDX = NPAD                # chunk-relative row of the first rank's zero row
MAX_IDX_CALL = 4096
MAX_NB = 16                  # max blocks per gather call (bounds reduce tile)
PB = [0, 24, 48, 73, 98]     # realign quarter-part block boundaries


def _wrap_idx(flat):
    """int16 flat gather list -> [128, len/16] SBUF wrap (16 partitions, x8)."""
    iw = len(flat) // 16
    w = flat.reshape(iw, 16).T
    return np.ascontiguousarray(np.tile(w, (8, 1)).astype(np.int16))


def _host_prep(in_feat, src, dst):
    deg = np.bincount(dst, minlength=N)
    dinv = (1.0 / np.sqrt(np.maximum(deg, 1))).astype(np.float32)

    core_of = dst // NL
    chunk_of = src // (2 * NL)
    idx16 = ((src // NL - 2 * chunk_of) * TRr + src % NL).astype(np.int32)

    key = core_of * NCHUNK + chunk_of
    order = np.argsort(key, kind="stable")
    bounds = np.searchsorted(key[order], np.arange(M * NCHUNK + 1))

    K = np.zeros((NCHUNK, NBLK), dtype=np.int64)
    groups = {}
    degc_all = np.zeros((M, NCHUNK, NPAD), dtype=np.int64)
    ords = np.empty((M, NCHUNK, NPAD), dtype=np.int64)
    lanes = np.empty((M, NCHUNK, NPAD), dtype=np.int32)
    for c in range(M):
        for c4 in range(NCHUNK):
            g = order[bounds[c * NCHUNK + c4] : bounds[c * NCHUNK + c4 + 1]]
            groups[(c, c4)] = g
            dl = dst[g] - c * NL
            dc = np.bincount(dl, minlength=NPAD)
            degc_all[c, c4] = dc
            o = np.argsort(-dc, kind="stable")
            ords[c, c4] = o
            inv = np.empty(NPAD, dtype=np.int32)
            inv[o] = np.arange(NPAD, dtype=np.int32)
            lanes[c, c4] = inv
            K[c4] = np.maximum(K[c4], dc[o].reshape(NBLK, NP)[:, 0])

    # call schedule: per chunk, runs of equal-K consecutive blocks, capped
    calls = []  # (c4, kb, b0, nb, nidx)
    for c4 in range(NCHUNK):
        b = 0
        while b < NBLK:
            kb = int(K[c4][b])
            if kb == 0:
                b += 1
                continue
            e_ = b
            while e_ + 1 < NBLK and int(K[c4][e_ + 1]) == kb:
                e_ += 1
            maxnb = min(MAX_NB, max(1, MAX_IDX_CALL // (NP * kb)))
            while b <= e_:
                nb = min(maxnb, e_ - b + 1)
                calls.append((c4, kb, b, nb, NP * kb * nb))
                b += nb

    chunk_cols = [0] * NCHUNK
    call_col = []
    for (c4, kb, b0, nb, nidx) in calls:
        call_col.append(chunk_cols[c4])
        chunk_cols[c4] += nidx // 16

    idx_inputs = []
    ridx_inputs = []
    for c in range(M):
        per_chunk = []
        for c4 in range(NCHUNK):
            g = groups[(c, c4)]
            dl = dst[g] - c * NL
            lane = lanes[c, c4][dl].astype(np.int64)
            eorder = np.argsort(lane, kind="stable")
            ge = g[eorder]
            lane_s = lane[eorder]
            counts = degc_all[c, c4][ords[c, c4]]
            starts = np.zeros(NPAD + 1, dtype=np.int64)
            np.cumsum(counts, out=starts[1:])
            slot = np.arange(len(ge)) - starts[lane_s]
            flat = np.full(chunk_cols[c4] * 16, PADIDX, dtype=np.int32)
            blk = lane_s // NP
            j = lane_s % NP
            for ci, (cc4, kb, b0, nb, nidx) in enumerate(calls):
                if cc4 != c4:
                    continue
                sel = (blk >= b0) & (blk < b0 + nb) & (slot < kb)
                base = call_col[ci] * 16
                pos = base + ((blk[sel] - b0) * kb + slot[sel]) * NP + j[sel]
                flat[pos] = idx16[ge[sel]]
            per_chunk.append(_wrap_idx(flat.astype(np.int16)))
        idx_inputs.append(per_chunk)
        rflat = np.concatenate(
            [lanes[c, c4][:NPAD].astype(np.int16) for c4 in range(NCHUNK)]
        )
        ridx_inputs.append(_wrap_idx(rflat))

    xt_in, dinv_in = [], []
    for c in range(M):
        xt = np.zeros((IN, NPAD), dtype=np.float32)
        xt[:, :NL] = in_feat[c * NL : (c + 1) * NL].T
        xt_in.append(np.ascontiguousarray(xt))
        dv = np.ones(NPAD, dtype=np.float32)
        dv[:NL] = dinv[c * NL : (c + 1) * NL]
        dinv_in.append(np.ascontiguousarray(dv.reshape(NBLK, NP).T))
    return calls, call_col, chunk_cols, K, idx_inputs, ridx_inputs, xt_in, dinv_in


def _weights(W1, b1, W2, b2, W3, b3, W4, b4):
    Mk = [
        sum(THETAS[t][k] * W3[:, t * H : (t + 1) * H] for t in range(len(THETAS)))
        for k in range(3)
    ]
    return {
        "W1t": np.ascontiguousarray(W1.T.astype(np.float32)),
        "W2t": np.ascontiguousarray(W2.T.astype(np.float32)),
        "M0t": np.ascontiguousarray(Mk[0].T.astype(np.float32)),
        "M1t": np.ascontiguousarray(Mk[1].T.astype(np.float32)),
        "M2t": np.ascontiguousarray(Mk[2].T.astype(np.float32)),
        "W4t": np.ascontiguousarray(W4.T.astype(np.float32)),
        "W2tb": np.ascontiguousarray(
            np.vstack([W2.T, b2.reshape(1, H)]).astype(np.float32)
        ),
        "W4tb": np.ascontiguousarray(
            np.vstack([W4.T, b4.reshape(1, C)]).astype(np.float32)
        ),
        "b1c": b1.reshape(H, 1).astype(np.float32),
        "b3c": b3.reshape(H, 1).astype(np.float32),
    }


def _build_program(calls, call_col, chunk_cols, K):
    import concourse.bacc as bacc
    import concourse.mybir as mybir
    import concourse.tile as tile
    from concourse.library_config import mlp
    from concourse.masks import make_identity

    f32 = mybir.dt.float32
    f32r = mybir.dt.float32r
    AF = mybir.ActivationFunctionType
    i16 = mybir.dt.int16
    AX = mybir.AxisListType
    OP = mybir.AluOpType

    nc = bacc.Bacc(
        "TRN2", target_bir_lowering=False, debug=False, num_devices=M,
        num_swdge_queues=4,
    )

    xt_ext = nc.declare_dram_parameter("xt", [IN, NPAD], f32r, isOutput=False)
    dinv_ext = nc.declare_dram_parameter("dinvT", [NP, NBLK], f32, isOutput=False)
    idx_ext = [
        nc.declare_dram_parameter(f"idx{c4}", [128, chunk_cols[c4]], i16, isOutput=False)
        for c4 in range(NCHUNK)
    ]
    ridx_ext = nc.declare_dram_parameter(
        "ridx", [128, NCHUNK * (NPAD // 16)], i16, isOutput=False
    )
    wshapes = [
        ("W1t", [IN, H], f32r), ("W2t", [H, H], f32r), ("M0t", [H, H], f32r),
        ("M1t", [H, H], f32r), ("M2t", [H, H], f32r), ("W4t", [H, C], f32r),
        ("W2tb", [H + 1, H], f32r), ("W4tb", [H + 1, C], f32r),
        ("b1c", [H, 1], f32), ("b3c", [H, 1], f32),
    ]
    wext = {nm: nc.declare_dram_parameter(nm, s, dt, isOutput=False) for nm, s, dt in wshapes}
    out_ext = nc.declare_dram_parameter("out", [C, NPAD], f32, isOutput=True)

    gloc = [nc.dram_tensor(f"g{h}loc", [TRr, H], f32) for h in range(2)]
    gfull = [
        nc.dram_tensor(f"g{h}full", [M * TRr, H], f32, addr_space="Shared")
        for h in range(2)
    ]
    aggdram = [nc.dram_tensor(f"agg{c4}", [NPAD, H], f32) for c4 in range(NCHUNK)]
    rg = [list(range(M))]

    with tile.TileContext(nc) as tc:
        with (
            tc.tile_pool(name="const", bufs=1) as cpool,
            tc.tile_pool(name="big", bufs=1) as bigpool,
            tc.tile_pool(name="xt", bufs=2) as xtpool,
            tc.tile_pool(name="work", bufs=2) as wpool,
            tc.tile_pool(name="idxp", bufs=2) as ipool,
            tc.tile_pool(name="gath", bufs=6) as gpool,
            tc.tile_pool(name="ridxp", bufs=3) as rpool,
            tc.tile_pool(name="ps", bufs=3, space="PSUM") as pspool,
        ):
            nc.gpsimd.load_library(mlp)

            W = {}
            for nm, s, dt in wshapes:
                W[nm] = cpool.tile(list(s), dt, tag=nm, name=nm)
                nc.sync.dma_start(out=W[nm][:], in_=wext[nm][:])
            dinvT = cpool.tile([NP, NBLK], f32, tag="dinvT")
            nc.sync.dma_start(out=dinvT[:], in_=dinv_ext[:])
            zrow = cpool.tile([1, H], f32, tag="zrow")
            nc.vector.memset(zrow[:], 0.0)
            zblk = cpool.tile([NP, H], f32, tag="zblk")
            nc.vector.memset(zblk[:], 0.0)
            ident = cpool.tile([NP, NP], f32, tag="ident")
            make_identity(nc, ident[:])
            ridx_t = cpool.tile([128, NCHUNK * (NPAD // 16)], i16, tag="ridx")
            nc.sync.dma_start(out=ridx_t[:], in_=ridx_ext[:])

            f0 = bigpool.tile([NP, NBLK * H], f32, tag="f0")
            f1 = bigpool.tile([NP, NBLK * H], f32, tag="f1")
            fX = bigpool.tile([NP, NBLK * H], f32, tag="fX")

            gloc_v = [g.ap()[0:NPAD, :].rearrange("(b j) d -> j b d", j=NP) for g in gloc]
            agg_v = [a.ap().rearrange("(b j) d -> j b d", j=NP) for a in aggdram]
            dbc = (
                dinvT[:]
                .rearrange("p (b o) -> p b o", o=1)
                .to_broadcast([NP, NBLK, H])
            )

            # ---------- phase A: L1 + L2 column-major, flip back per block ----
            GA = 4
            for g4 in range(0, NBLK, GA):
                nbg = min(GA, NBLK - g4)
                wg = nbg * NP
                xt = xtpool.tile([IN, GA * NP], f32r, tag="xt")
                nc.sync.dma_start(
                    out=xt[:, :wg], in_=xt_ext[:, g4 * NP : g4 * NP + wg]
                )
                ps1 = pspool.tile([NP, 512], f32, tag="pS", space="PSUM")
                nc.tensor.matmul(out=ps1[:H, :wg], lhsT=W["W1t"][:], rhs=xt[:, :wg], start=True, stop=True)
                h1X = wpool.tile([H + 1, GA * NP], f32r, tag="h1T")
                nc.scalar.activation(
                    h1X[:H, :wg], ps1[:H, :wg], AF.Relu, bias=W["b1c"][:, 0:1],
                )
                nc.vector.memset(h1X[H : H + 1, :wg].bitcast(f32), 1.0)
                gb = wpool.tile([NP, 25 * H], f32, tag="gbh")
                for i in range(nbg):
                    b = g4 + i
                    ps2b = pspool.tile([NP, NP], f32, tag="pA", space="PSUM")
                    nc.tensor.matmul(
                        out=ps2b[:, :H], lhsT=h1X[:, i * NP : (i + 1) * NP],
                        rhs=W["W2tb"][:], start=True, stop=True,
                    )
                    nc.vector.tensor_scalar(
                        out=f0[:, b * H : (b + 1) * H], in0=ps2b[:, :H],
                        scalar1=0.0, scalar2=None, op0=OP.max,
                    )
                dbc_s = (
                    dinvT[:, g4 : g4 + nbg]
                    .rearrange("p (b o) -> p b o", o=1)
                    .to_broadcast([NP, nbg, H])
                )
                nc.vector.tensor_tensor(
                    out=gb[:, : nbg * H].rearrange("p (b d) -> p b d", b=nbg),
                    in0=f0[:, g4 * H : (g4 + nbg) * H].rearrange("p (b d) -> p b d", b=nbg),
                    in1=dbc_s, op=OP.mult,
                )
                nc.scalar.dma_start(
                    out=gloc_v[0][:, g4 : g4 + nbg, :],
                    in_=gb[:, : nbg * H].rearrange("p (b d) -> p b d", b=nbg),
                )
            nc.sync.dma_start(out=gloc[0][NPAD : NPAD + 1, :], in_=zrow[:])

            nc.gpsimd.collective_compute(
                "AllGather", OP.bypass, replica_groups=rg,
                ins=[gloc[0].ap().opt()], outs=[gfull[0].ap().opt()],
            )

            MAXCC = max(chunk_cols)
            by_chunk = [[] for _ in range(NCHUNK)]
            for ci, cl in enumerate(calls):
                by_chunk[cl[0]].append((ci, cl))

            # ---------- two hops ----------
            for h in range(2):
                fprev = f0 if h == 0 else f1
                fout = f1 if h == 0 else fX
                table = gfull[h]

                def emit_realign(c4, first, last):
                    for p in range(4):
                        pb0, pb1 = PB[p], PB[p + 1]
                        npb = pb1 - pb0
                        rt = rpool.tile([NP, 25, H], f32, tag="rt")
                        coff = c4 * (NPAD // 16) + pb0 * (NP // 16)
                        nc.gpsimd.dma_gather(
                            rt[:, :npb, :],
                            aggdram[c4][:, :],
                            ridx_t[:, coff : coff + npb * (NP // 16)],
                            npb * NP,
                            npb * NP,
                            H,
                            single_packet=False,
                            queue_num=p,
                        )
                        rtf = rt[:, :npb, :].rearrange("p b d -> p (b d)")
                        fxs = fX[:, pb0 * H : pb1 * H]
                        if first:
                            nc.vector.tensor_copy(fxs, rtf)
                        else:
                            nc.vector.tensor_tensor(out=fxs, in0=fxs, in1=rtf, op=OP.add)
                        if last:
                            # fused epilogue for this block range
                            dbc_p = (
                                dinvT[:, pb0:pb1]
                                .rearrange("p (b o) -> p b o", o=1)
                                .to_broadcast([NP, npb, H])
                            )
                            fx3 = fxs.rearrange("p (b d) -> p b d", b=npb)
                            nc.vector.tensor_tensor(out=fx3, in0=fx3, in1=dbc_p, op=OP.mult)
                            fo = fout[:, pb0 * H : pb1 * H]
                            nc.vector.tensor_tensor(
                                out=fo, in0=fprev[:, pb0 * H : pb1 * H], in1=fxs,
                                op=OP.subtract,
                            )
                            if h == 0:
                                gb = wpool.tile([NP, 25 * H], f32, tag="gbh")
                                nc.vector.tensor_tensor(
                                    out=gb[:, : npb * H].rearrange("p (b d) -> p b d", b=npb),
                                    in0=fo.rearrange("p (b d) -> p b d", b=npb),
                                    in1=dbc_p, op=OP.mult,
                                )
                                nc.scalar.dma_start(
                                    out=gloc_v[1][:, pb0:pb1, :],
                                    in_=gb[:, : npb * H].rearrange("p (b d) -> p b d", b=npb),
                                )

                qrr = 0
                order4 = sorted(range(NCHUNK), key=lambda q: -chunk_cols[q])
                for pos in range(NCHUNK):
                    c4 = order4[pos]
                    it = ipool.tile([128, MAXCC], i16, tag="idxc")
                    nc.sync.dma_start(
                        out=it[:, : chunk_cols[c4]], in_=idx_ext[c4][:]
                    )
                    for ci, (cc4, kb, b0, nb, nidx) in by_chunk[c4]:
                        S = nidx // NP
                        dst_t = gpool.tile([NP, MAX_IDX_CALL // NP, H], f32, tag="gdst")
                        nc.gpsimd.dma_gather(
                            dst_t[:, :S, :],
                            table[c4 * CHROWS : (c4 + 1) * CHROWS, :],
                            it[:, call_col[ci] : call_col[ci] + nidx // 16],
                            nidx,
                            nidx,
                            H,
                            single_packet=False,
                            queue_num=qrr % 4,
                        )
                        qrr += 1
                        red = wpool.tile([NP, MAX_NB * H], f32, tag="red")
                        nc.vector.tensor_reduce(
                            out=red[:, : nb * H].rearrange("p (b d) -> p b d", b=nb),
                            in_=dst_t[:, :S, :].rearrange("p (b k) d -> p b d k", b=nb, k=kb),
                            axis=AX.X,
                            op=OP.add,
                        )
                        nc.scalar.dma_start(
                            out=agg_v[c4][:, b0 : b0 + nb, :],
                            in_=red[:, : nb * H].rearrange("p (b d) -> p b d", b=nb),
                        )
                    for b in range(NBLK):
                        if K[c4][b] == 0:
                            nc.sync.dma_start(
                                out=aggdram[c4][b * NP : (b + 1) * NP, :], in_=zblk[:]
                            )
                    if pos >= 2:
                        emit_realign(order4[pos - 2], pos - 2 == 0, False)
                emit_realign(order4[NCHUNK - 2], False, False)
                emit_realign(order4[NCHUNK - 1], False, True)

                if h == 0:
                    nc.sync.dma_start(out=gloc[1][NPAD : NPAD + 1, :], in_=zrow[:])
                    nc.gpsimd.collective_compute(
                        "AllGather", OP.bypass, replica_groups=rg,
                        ins=[gloc[1].ap().opt()], outs=[gfull[1].ap().opt()],
                    )

            # ---------- phase E: L3 + L4 column-major ----------
            f2 = fX
            for t in range(0, NBLK, 4):
                nbg = min(4, NBLK - t)
                w = nbg * NP
                ps3 = pspool.tile([NP, 512], f32, tag="pS", space="PSUM")
                for k, (fk, mk) in enumerate(zip((f0, f1, f2), ("M0t", "M1t", "M2t"))):
                    fkT = wpool.tile([H, 512], f32r, tag="fkT")
                    for pr in range((nbg + 1) // 2):
                        nbp = min(2, nbg - 2 * pr)  # blocks in this pair
                        psT = pspool.tile([NP, NP], f32, tag="pA", space="PSUM")
                        nc.tensor.transpose(
                            out=psT[: nbp * H, :],
                            in_=fk[:, (t + 2 * pr) * H : (t + 2 * pr + nbp) * H],
                            identity=ident[:],
                        )
                        nc.vector.tensor_copy(
                            fkT[:, 2 * pr * NP : 2 * pr * NP + NP], psT[0:H, :NP]
                        )
                        if nbp > 1:
                            nc.scalar.copy(
                                fkT[:, (2 * pr + 1) * NP : (2 * pr + 2) * NP],
                                psT[H : 2 * H, :NP],
                            )
                    nc.tensor.matmul(
                        out=ps3[:H, :w], lhsT=W[mk][:], rhs=fkT[:, :w],
                        start=(k == 0), stop=(k == 2),
                    )
                h3X = wpool.tile([H + 1, 512], f32r, tag="h3T")
                nc.scalar.activation(
                    h3X[:H, :w], ps3[:H, :w], AF.Relu, bias=W["b3c"][:, 0:1],
                )
                nc.vector.memset(h3X[H : H + 1, :w].bitcast(f32), 1.0)
                psO = pspool.tile([NP, 512], f32, tag="pS", space="PSUM")
                nc.tensor.matmul(out=psO[:C, :w], lhsT=W["W4tb"][:], rhs=h3X[:, :w], start=True, stop=True)
                oT = wpool.tile([C, 512], f32, tag="oT")
                nc.vector.tensor_copy(oT[:, :w], psO[:C, :w])
                nc.sync.dma_start(out=out_ext[:, t * NP : t * NP + w], in_=oT[:, :w])

    nc.compile()
    return nc


def kernel(**inputs):
    import concourse.bass_utils as bass_utils

    in_feat = np.asarray(inputs["in_feat"], dtype=np.float32)
    src = np.asarray(inputs["src"]).astype(np.int64)
    dst = np.asarray(inputs["dst"]).astype(np.int64)

    (calls, call_col, chunk_cols, K, idx_inputs, ridx_inputs, xt_in, dinv_in) = (
        _host_prep(in_feat, src, dst)
    )
    weights = _weights(
        np.asarray(inputs["W1"]), np.asarray(inputs["b1"]),
        np.asarray(inputs["W2"]), np.asarray(inputs["b2"]),
        np.asarray(inputs["W3"]), np.asarray(inputs["b3"]),
        np.asarray(inputs["W4"]), np.asarray(inputs["b4"]),
    )

    nc = _build_program(calls, call_col, chunk_cols, K)

    in_maps = []
    for c in range(M):
        im = {"xt": xt_in[c], "dinvT": dinv_in[c], "ridx": ridx_inputs[c]}
        for c4 in range(NCHUNK):
            im[f"idx{c4}"] = idx_inputs[c][c4]
        im.update(weights)
        in_maps.append(im)

    trace = bool(int(os.environ.get("BWGNN_TRACE", "0")))
    res = bass_utils.run_bass_kernel_spmd(nc, in_maps, list(range(M)), trace=trace)
    global LAST_EXEC_NS
    LAST_EXEC_NS = res.exec_time_ns

    full = np.empty((N, C), dtype=np.float32)
    for c in range(M):
        r = res.results[c]["out"]  # [C, NPAD]
        full[c * NL : (c + 1) * NL] = r[:, :NL].T
    return full
